# revision 25
# baseline (speedup 1.0000x reference)
"""Bahdanau additive attention on 8 Trainium2 NeuronCores.

Per-core program (data-parallel over batch B=8, one batch element per core):
  qT[h,q]   = Wq @ query.T + (bq+bk)       h on partitions (2 chunks of 128)
  kT[h,m]   = Wk @ memory.T                h on partitions, PSUM-resident
  z[h,m]    = tanh(kT + qT[:,q])           ACT, bias = per-partition qT column
  logits[q,m] = sum_h wl[h] * z[h,m]       PE, M=32-replicated matmuls into
                                           PSUM col-group rows {0,32,64,96}
  weights   = softmax_m(logits + mask*-1e18)
  attns     = weights @ memory

Work split across engines (per core, per q-row): ACT computes tanh for
hc=0 and for 128-N_DVE_JOBS of the hc=1 jobs; for the rest, GPSIMD computes
u = e^{2qT}*e^{2kT}+1 and the DVE a fast approximate reciprocal r=1/u
(tanh = 1-2r up to a per-row constant that softmax cancels), so all three
elementwise engines run the hot loop concurrently. The additive bias `bl`
shifts every logit uniformly -> softmax-invariant -> dropped. The mask is
applied as a -1e18 additive fill before exp, exactly like the reference.
"""

import os
from contextlib import ExitStack

import numpy as np

import concourse.bass as bass
import concourse.bacc as bacc
import concourse.tile as tile
from concourse import mybir
from concourse.masks import make_identity

B, QL, ML = 8, 128, 1024
QS, KS, H = 512, 512, 256
N_CORES = 8

FP32 = mybir.dt.float32
BF16 = mybir.dt.bfloat16
U8 = mybir.dt.uint8

SC = QS // 128  # 4 s-chunks
HC = H // 128   # 2 h-chunks
MT = ML // 128  # 8 m-tiles

# v2: offload this many of the 128 (q, hc=1) tanh jobs from ACT to DVE via
# tanh(x) = 1 - 2/(e^{2q}e^{2k}+1)  (the per-row constant sum(wl) cancels in
# softmax). 0 disables the DVE path entirely.
N_DVE_JOBS = int(os.environ.get("BAHDANAU_DVE_JOBS", "120"))
# of those, how many u=A*B+1 passes run on GPSIMD instead of DVE
N_GPS_U = int(os.environ.get("BAHDANAU_GPS_U", "120"))
# of the 32 region evacuations, how many run on ACT instead of DVE
N_ACT_EVAC = int(os.environ.get("BAHDANAU_ACT_EVAC", "8"))
FP32R = mybir.dt.float32r


def build_kernel():
    nc = bacc.Bacc(None, target_bir_lowering=False)

    # ---- DRAM parameters (per-core slices supplied host-side) ----
    query = nc.declare_dram_parameter("query", [QL, QS], FP32, isOutput=False)
    memory = nc.declare_dram_parameter("memory", [ML, KS], FP32, isOutput=False)
    mask = nc.declare_dram_parameter("mask", [ML], U8, isOutput=False)
    wqt = nc.declare_dram_parameter("wqt", [QS, H], FP32, isOutput=False)
    wkt = nc.declare_dram_parameter("wkt", [KS, H], FP32, isOutput=False)
    bqk = nc.declare_dram_parameter("bqk", [128, HC], FP32, isOutput=False)
    wlrep = nc.declare_dram_parameter("wlrep", [128, HC, 32], FP32, isOutput=False)
    wlneg = nc.declare_dram_parameter("wlneg", [128, 32], FP32, isOutput=False)
    attns_o = nc.declare_dram_parameter("attns", [QL, QS], FP32, isOutput=True)
    weights_o = nc.declare_dram_parameter("weights", [QL, ML], FP32, isOutput=True)

    with ExitStack() as ctx:
        tc = ctx.enter_context(tile.TileContext(nc))
        const = ctx.enter_context(tc.tile_pool(name="const", bufs=1))
        sb = ctx.enter_context(tc.tile_pool(name="sb", bufs=1))
        zpool = ctx.enter_context(tc.tile_pool(name="z", bufs=6))
        # PSUM stack: kT (4 banks) lives for the whole kernel.
        psum_kt = ctx.enter_context(tc.tile_pool(name="psum_kt", bufs=1, space="PSUM"))

        # ---- constant / input loads ----
        ident = const.tile([128, 128], BF16)
        make_identity(nc, ident)

        # bf16 casting loads (gpsimd SWDGE does dtype conversion)
        q_bf = sb.tile([128, QS], BF16)               # [q, s]
        nc.gpsimd.dma_start(out=q_bf, in_=query[:])
        mem_bf = sb.tile([128, MT, KS], BF16)         # [m_p, m_t, s]
        mem_r = memory[:].rearrange("(t p) s -> p t s", p=128)
        for mt in range(MT):
            nc.gpsimd.dma_start(out=mem_bf[:, mt, :], in_=mem_r[:, mt, :])
        wqt_bf = sb.tile([128, SC, H], BF16)          # [s_p, s_c, h]
        nc.gpsimd.dma_start(
            out=wqt_bf, in_=wqt[:].rearrange("(c p) h -> p c h", p=128)
        )
        wkt_bf = sb.tile([128, SC, H], BF16)
        nc.gpsimd.dma_start(
            out=wkt_bf, in_=wkt[:].rearrange("(c p) h -> p c h", p=128)
        )
        bqk_sb = const.tile([128, HC], FP32)          # [h_p, h_c]
        nc.gpsimd.dma_start(out=bqk_sb, in_=bqk[:])
        # wl replicated 32x along free dim (host-side layout): lhsT [128, 32]
        # per h-chunk, so M=32 matmuls fill a whole PE col-group (same cost
        # as M=1).
        wl_bf = const.tile([128, HC, 32], BF16)       # [h_p, h_c, rep]
        nc.gpsimd.dma_start(out=wl_bf, in_=wlrep[:])

        mask_u8 = sb.tile([128, ML], U8)
        m_ap = mask[:]
        nc.gpsimd.dma_start(
            out=mask_u8,
            in_=bass.AP(tensor=m_ap.tensor, offset=m_ap.offset,
                        ap=[[0, 128]] + list(m_ap.ap)),
        )
        maskb = sb.tile([128, ML], FP32)              # -1e18 at masked, 0 else
        nc.vector.tensor_scalar(
            out=maskb, in0=mask_u8, scalar1=-1e18, scalar2=None,
            op0=mybir.AluOpType.mult,
        )

        # kT PSUM-resident: 2 x [h_p, m] fp32 = 4 banks (separate tiles so
        # hc=0 consumers don't falsely depend on hc=1 writers)
        kt_ps = [psum_kt.tile([128, ML], FP32, tag=f"kt{hc}", name=f"kt{hc}")
                 for hc in range(HC)]

        qTb = [sb.tile([128, QL], FP32, tag=f"qTb{hc}", name=f"qTb{hc}")
               for hc in range(HC)]

        # ---- preamble: transposes + projections (scoped PSUM pool) ----
        with tc.tile_pool(name="psum_tr", bufs=2, space="PSUM") as trp:
            # query^T: [s_p, s_c, q]
            qT_bf = sb.tile([128, SC, QL], BF16)
            for sc in range(SC):
                pt = trp.tile([128, 128], BF16, tag="tr")
                nc.tensor.transpose(pt, q_bf[:, sc * 128:(sc + 1) * 128], ident)
                nc.vector.tensor_copy(out=qT_bf[:, sc, :], in_=pt)
            # memory^T: [s_p, s_c, m]  (mt-outer so the kT projection's
            # first m-half can start after 4 m-tiles are transposed)
            memT_bf = sb.tile([128, SC, ML], BF16)
            for mt in range(MT):
                for sc in range(SC):
                    pt = trp.tile([128, 128], BF16, tag="tr")
                    nc.tensor.transpose(
                        pt, mem_bf[:, mt, sc * 128:(sc + 1) * 128], ident
                    )
                    nc.scalar.copy(
                        out=memT_bf[:, sc, mt * 128:(mt + 1) * 128], in_=pt
                    )

            # qT projection: accumulate over s-chunks
            for hc in range(HC):
                pq = trp.tile([128, QL], FP32, tag="pq")
                for sc in range(SC):
                    nc.tensor.matmul(
                        pq,
                        wqt_bf[:, sc, hc * 128:(hc + 1) * 128],
                        qT_bf[:, sc, :],
                        start=(sc == 0),
                        stop=(sc == SC - 1),
                    )
                # qTb = pq + (bq+bk), per-partition scalar add
                nc.vector.tensor_scalar(
                    out=qTb[hc], in0=pq,
                    scalar1=bqk_sb[:, hc:hc + 1], scalar2=None,
                    op0=mybir.AluOpType.add,
                )

            # kT projection straight into resident PSUM
            for mh in range(2):
                for hc in range(HC):
                    out_sl = kt_ps[hc][:, mh * 512:(mh + 1) * 512]
                    for sc in range(SC):
                        nc.tensor.matmul(
                            out_sl,
                            wkt_bf[:, sc, hc * 128:(hc + 1) * 128],
                            memT_bf[:, sc, mh * 512:(mh + 1) * 512],
                            start=(sc == 0),
                            stop=(sc == SC - 1),
                        )

        # ---- v2: DVE tanh offload setup ----
        # For a subset of (q, hc=1) jobs, DVE computes r = 1/(e^{2q}e^{2k}+1)
        # and the PE dot uses weights -2*wl on r; tanh = 1-2r up to the
        # per-row constant sum(wl) which softmax cancels.
        dve_q = set()
        if N_DVE_JOBS > 0:
            step = QL / N_DVE_JOBS
            dve_q = {int(i * step) for i in range(N_DVE_JOBS)}
        gps_q = set()
        if N_GPS_U > 0 and dve_q:
            dl = sorted(dve_q)
            stepg = len(dl) / N_GPS_U
            gps_q = {dl[int(i * stepg)] for i in range(N_GPS_U)}
        if dve_q:
            A1 = sb.tile([128, QL], FP32)        # e^{2*qTb[hc=1]}
            nc.scalar.activation(out=A1, in_=qTb[1],
                                 func=mybir.ActivationFunctionType.Exp,
                                 scale=2.0)
            Bx1 = sb.tile([128, ML], FP32)       # e^{2*kT[hc=1]}
            nc.scalar.activation(out=Bx1, in_=kt_ps[1],
                                 func=mybir.ActivationFunctionType.Exp,
                                 scale=2.0)
            wlneg_sb = const.tile([128, 32], BF16)  # -2*wl hc=1, replicated
            nc.gpsimd.dma_start(out=wlneg_sb, in_=wlneg[:])
            upool = ctx.enter_context(tc.tile_pool(name="u", bufs=3))
            rpool = ctx.enter_context(tc.tile_pool(name="r", bufs=6))

        # ---- main loop ----
        # Matmul PSUM outputs may only start at partitions {0,32,64,96}
        # (PE col-groups), so queries go 4-per-PSUM-region at those rows,
        # get evacuated to SBUF staging, and a small SBUF->SBUF DMA gathers
        # the 4 rows onto consecutive partitions of the logits tile.
        logits_sb = sb.tile([128, ML], FP32)
        stg_pool = ctx.enter_context(tc.tile_pool(name="stg", bufs=4))

        with tc.tile_pool(name="psum_rg", bufs=2, space="PSUM") as rgp:
            for g in range(QL // 4):
                rg = rgp.tile([128, ML], FP32, tag="rg")
                z0s, h1s = [], []
                for qc in range(4):
                    q = 4 * g + qc
                    z0 = zpool.tile([128, ML], BF16, tag="z0")
                    nc.scalar.activation(
                        out=z0, in_=kt_ps[0],
                        func=mybir.ActivationFunctionType.Tanh,
                        bias=qTb[0][:, q:q + 1], scale=1.0,
                    )
                    z0s.append(z0)
                from concourse.dve_ops import (
                    RECIP_APPROX_FAST_CONSTS as _RC,
                    RECIPROCAL_APPROX_FAST as _RF,
                )
                for qc in range(4):
                    q = 4 * g + qc
                    if q in dve_q:
                        ut = upool.tile([128, ML], FP32, tag="u")
                        ueng = nc.gpsimd if q in gps_q else nc.vector
                        ueng.tensor_scalar(
                            out=ut, in0=Bx1, scalar1=A1[:, q:q + 1],
                            scalar2=1.0, op0=mybir.AluOpType.mult,
                            op1=mybir.AluOpType.add,
                        )
                        # custom-DVE recip, bf16 out (the fp32 restriction
                        # is about the BITWISE_NOT on the *input*; output
                        # goes through the normal dtype converter) - saves
                        # a separate cast pass.
                        rt = rpool.tile([128, ML], BF16, tag="r")
                        nc.vector._custom_dve(
                            _RF, out=rt[:], in0=ut[:],
                            s0=_RC["s0"], s1=_RC["s1"], imm2=_RC["imm2"],
                        )
                        h1s.append((rt, wlneg_sb))
                    else:
                        z1 = rpool.tile([128, ML], BF16, tag="r")
                        nc.scalar.activation(
                            out=z1, in_=kt_ps[1],
                            func=mybir.ActivationFunctionType.Tanh,
                            bias=qTb[1][:, q:q + 1], scale=1.0,
                        )
                        h1s.append((z1, None))
                for qc in range(4):
                    rt, wneg = h1s[qc]
                    for mh in range(2):
                        sl = slice(mh * 512, (mh + 1) * 512)
                        nc.tensor.matmul(
                            rg[32 * qc:32 * qc + 32, sl],
                            wl_bf[:, 0, :],
                            z0s[qc][:, sl],
                            start=True, stop=False,
                            tile_position=(0, 32 * qc),
                        )
                        nc.tensor.matmul(
                            rg[32 * qc:32 * qc + 32, sl],
                            wneg[:] if wneg is not None else wl_bf[:, 1, :],
                            rt[:, sl],
                            start=False, stop=True,
                            tile_position=(0, 32 * qc),
                        )
                stage = stg_pool.tile([128, ML], FP32, tag="stage")
                st_ap = stage[:]
                if N_ACT_EVAC and g % max(1, 32 // N_ACT_EVAC) == 0:
                    nc.scalar.copy(out=stage, in_=rg)
                else:
                    nc.vector.tensor_copy(out=stage, in_=rg)
                p_stride = st_ap.ap[0][0]
                nc.sync.dma_start(
                    out=logits_sb[4 * g:4 * g + 4, :],
                    in_=bass.AP(tensor=st_ap.tensor, offset=st_ap.offset,
                                ap=[[32 * p_stride, 4]] + list(st_ap.ap)[1:]),
                )

        psum_at = ctx.enter_context(tc.tile_pool(name="psum_at", bufs=1, space="PSUM"))

        # ---- masked softmax over m (rows = q on partitions) ----
        # |logits| <= H*max|wl| = 16, so exp() is safe without the rowmax
        # shift; softmax is shift-invariant so the result is identical.
        # The DVE mask-add also stands between the 32 gather DMAs and the
        # ACT exp: an ACT instruction waiting directly on that many DMA
        # queues hangs the exec unit (observed NRT_EXEC_UNIT_UNRECOVERABLE).
        lmask = sb.tile([128, ML], FP32)
        nc.vector.tensor_add(lmask, logits_sb, maskb)
        ewm = sb.tile([128, ML], FP32)
        rsum = sb.tile([128, 1], FP32)
        nc.scalar.activation(
            out=ewm, in_=lmask, func=mybir.ActivationFunctionType.Exp,
            scale=1.0,
        )
        nc.vector.tensor_reduce(
            out=rsum, in_=ewm, axis=mybir.AxisListType.X,
            op=mybir.AluOpType.add,
        )
        rinv = sb.tile([128, 1], FP32)
        nc.vector.reciprocal(out=rinv, in_=rsum)

        wout = sb.tile([128, ML], FP32)
        nc.gpsimd.tensor_scalar(
            out=wout, in0=ewm, scalar1=rinv, scalar2=None,
            op0=mybir.AluOpType.mult,
        )
        nc.sync.dma_start(out=weights_o[:], in_=wout)

        # ---- attns = (ewm @ memory) * rinv ----
        ewm_bf = sb.tile([128, ML], BF16)
        nc.gpsimd.tensor_copy(out=ewm_bf, in_=ewm)
        wT_sb = sb.tile([128, MT, 128], BF16)         # ewm^T: [m_p, m_t, q]
        for mt in range(MT):
            pt = psum_at.tile([128, 128], BF16, tag="wt")
            nc.tensor.transpose(pt, ewm_bf[:, mt * 128:(mt + 1) * 128], ident)
            nc.vector.tensor_copy(out=wT_sb[:, mt, :], in_=pt)
        pa = psum_at.tile([128, QS], FP32, tag="pa")
        for mt in range(MT):
            nc.tensor.matmul(
                pa, wT_sb[:, mt, :], mem_bf[:, mt, :],
                start=(mt == 0), stop=(mt == MT - 1),
            )
        attns_sb = sb.tile([128, QS], FP32)
        nc.vector.tensor_scalar(
            out=attns_sb, in0=pa, scalar1=rinv, scalar2=None,
            op0=mybir.AluOpType.mult,
        )
        nc.sync.dma_start(out=attns_o[:], in_=attns_sb)

    nc.compile()
    return nc


_NC_CACHE = None


def _get_nc():
    global _NC_CACHE
    if _NC_CACHE is None:
        _NC_CACHE = build_kernel()
    return _NC_CACHE


def make_in_maps(inputs):
    wqt = np.ascontiguousarray(np.asarray(inputs["Wq"], np.float32).T)
    wkt = np.ascontiguousarray(np.asarray(inputs["Wk"], np.float32).T)
    bqk_v = (np.asarray(inputs["bq"], np.float32)
             + np.asarray(inputs["bk"], np.float32))
    bqk = np.ascontiguousarray(bqk_v.reshape(HC, 128).T)
    wl_v = np.asarray(inputs["wl"], np.float32)
    wlrep = np.ascontiguousarray(
        np.repeat(wl_v.reshape(HC, 128).T[:, :, None], 32, axis=2))
    wlneg = np.ascontiguousarray(
        np.repeat(-2.0 * wl_v[128:256][:, None], 32, axis=1))
    in_maps = []
    for i in range(N_CORES):
        in_maps.append({
            "query": np.ascontiguousarray(inputs["query"][i], np.float32),
            "memory": np.ascontiguousarray(inputs["memory"][i], np.float32),
            "mask": np.ascontiguousarray(inputs["mask"][i]).astype(np.uint8),
            "wqt": wqt,
            "wkt": wkt,
            "bqk": bqk,
            "wlrep": wlrep,
            "wlneg": wlneg,
        })
    return in_maps


def kernel(**inputs):
    from concourse.bass_utils import run_bass_kernel_spmd

    nc = _get_nc()
    in_maps = make_in_maps(inputs)
    res = run_bass_kernel_spmd(nc, in_maps, list(range(N_CORES)))
    attns = np.stack([res.results[i]["attns"] for i in range(N_CORES)])
    weights = np.stack([res.results[i]["weights"] for i in range(N_CORES)])
    return attns, weights


# revision 26
# speedup vs baseline: 1.0168x; 1.0168x over previous
"""Bahdanau additive attention on 8 Trainium2 NeuronCores.

Per-core program (data-parallel over batch B=8, one batch element per core):
  qT[h,q]   = Wq @ query.T + (bq+bk)       h on partitions (2 chunks of 128)
  kT[h,m]   = Wk @ memory.T                h on partitions, PSUM-resident
  z[h,m]    = tanh(kT + qT[:,q])           ACT, bias = per-partition qT column
  logits[q,m] = sum_h wl[h] * z[h,m]       PE, M=32-replicated matmuls into
                                           PSUM col-group rows {0,32,64,96}
  weights   = softmax_m(logits + mask*-1e18)
  attns     = weights @ memory

Work split across engines (per core, per q-row): ACT computes tanh for
hc=0 and for 128-N_DVE_JOBS of the hc=1 jobs; for the rest, GPSIMD computes
u = e^{2qT}*e^{2kT}+1 and the DVE a fast approximate reciprocal r=1/u
(tanh = 1-2r up to a per-row constant that softmax cancels), so all three
elementwise engines run the hot loop concurrently. The additive bias `bl`
shifts every logit uniformly -> softmax-invariant -> dropped. The mask is
applied as a -1e18 additive fill before exp, exactly like the reference.
"""

import os
from contextlib import ExitStack

import numpy as np

import concourse.bass as bass
import concourse.bacc as bacc
import concourse.tile as tile
from concourse import mybir
from concourse.masks import make_identity

B, QL, ML = 8, 128, 1024
QS, KS, H = 512, 512, 256
N_CORES = 8

FP32 = mybir.dt.float32
BF16 = mybir.dt.bfloat16
U8 = mybir.dt.uint8

SC = QS // 128  # 4 s-chunks
HC = H // 128   # 2 h-chunks
MT = ML // 128  # 8 m-tiles

# v2: offload this many of the 128 (q, hc=1) tanh jobs from ACT to DVE via
# tanh(x) = 1 - 2/(e^{2q}e^{2k}+1)  (the per-row constant sum(wl) cancels in
# softmax). 0 disables the DVE path entirely.
N_DVE_JOBS = int(os.environ.get("BAHDANAU_DVE_JOBS", "117"))
# of those, how many u=A*B+1 passes run on GPSIMD instead of DVE
N_GPS_U = int(os.environ.get("BAHDANAU_GPS_U", "117"))
# of the 32 region evacuations, how many run on ACT instead of DVE
N_ACT_EVAC = int(os.environ.get("BAHDANAU_ACT_EVAC", "8"))
FP32R = mybir.dt.float32r


def build_kernel():
    nc = bacc.Bacc(None, target_bir_lowering=False)

    # ---- DRAM parameters (per-core slices supplied host-side) ----
    query = nc.declare_dram_parameter("query", [QL, QS], FP32, isOutput=False)
    memory = nc.declare_dram_parameter("memory", [ML, KS], FP32, isOutput=False)
    mask = nc.declare_dram_parameter("mask", [ML], U8, isOutput=False)
    wqt = nc.declare_dram_parameter("wqt", [QS, H], FP32, isOutput=False)
    wkt = nc.declare_dram_parameter("wkt", [KS, H], FP32, isOutput=False)
    bqk = nc.declare_dram_parameter("bqk", [128, HC], FP32, isOutput=False)
    wlrep = nc.declare_dram_parameter("wlrep", [128, HC, 32], FP32, isOutput=False)
    wlneg = nc.declare_dram_parameter("wlneg", [128, 32], FP32, isOutput=False)
    attns_o = nc.declare_dram_parameter("attns", [QL, QS], FP32, isOutput=True)
    weights_o = nc.declare_dram_parameter("weights", [QL, ML], FP32, isOutput=True)

    with ExitStack() as ctx:
        tc = ctx.enter_context(tile.TileContext(nc))
        const = ctx.enter_context(tc.tile_pool(name="const", bufs=1))
        sb = ctx.enter_context(tc.tile_pool(name="sb", bufs=1))
        zpool = ctx.enter_context(tc.tile_pool(name="z", bufs=6))
        # PSUM stack: kT (4 banks) lives for the whole kernel.
        psum_kt = ctx.enter_context(tc.tile_pool(name="psum_kt", bufs=1, space="PSUM"))

        # ---- constant / input loads ----
        ident = const.tile([128, 128], BF16)
        make_identity(nc, ident)

        # bf16 casting loads (gpsimd SWDGE does dtype conversion)
        q_bf = sb.tile([128, QS], BF16)               # [q, s]
        nc.gpsimd.dma_start(out=q_bf, in_=query[:])
        mem_bf = sb.tile([128, MT, KS], BF16)         # [m_p, m_t, s]
        mem_r = memory[:].rearrange("(t p) s -> p t s", p=128)
        for mt in range(MT):
            nc.gpsimd.dma_start(out=mem_bf[:, mt, :], in_=mem_r[:, mt, :])
        wqt_bf = sb.tile([128, SC, H], BF16)          # [s_p, s_c, h]
        nc.gpsimd.dma_start(
            out=wqt_bf, in_=wqt[:].rearrange("(c p) h -> p c h", p=128)
        )
        wkt_bf = sb.tile([128, SC, H], BF16)
        nc.gpsimd.dma_start(
            out=wkt_bf, in_=wkt[:].rearrange("(c p) h -> p c h", p=128)
        )
        bqk_sb = const.tile([128, HC], FP32)          # [h_p, h_c]
        nc.gpsimd.dma_start(out=bqk_sb, in_=bqk[:])
        # wl replicated 32x along free dim (host-side layout): lhsT [128, 32]
        # per h-chunk, so M=32 matmuls fill a whole PE col-group (same cost
        # as M=1).
        wl_bf = const.tile([128, HC, 32], BF16)       # [h_p, h_c, rep]
        nc.gpsimd.dma_start(out=wl_bf, in_=wlrep[:])

        mask_u8 = sb.tile([128, ML], U8)
        m_ap = mask[:]
        nc.gpsimd.dma_start(
            out=mask_u8,
            in_=bass.AP(tensor=m_ap.tensor, offset=m_ap.offset,
                        ap=[[0, 128]] + list(m_ap.ap)),
        )
        maskb = sb.tile([128, ML], FP32)              # -1e18 at masked, 0 else
        nc.vector.tensor_scalar(
            out=maskb, in0=mask_u8, scalar1=-1e18, scalar2=None,
            op0=mybir.AluOpType.mult,
        )

        # kT PSUM-resident: 2 x [h_p, m] fp32 = 4 banks (separate tiles so
        # hc=0 consumers don't falsely depend on hc=1 writers)
        kt_ps = [psum_kt.tile([128, ML], FP32, tag=f"kt{hc}", name=f"kt{hc}")
                 for hc in range(HC)]

        qTb = [sb.tile([128, QL], FP32, tag=f"qTb{hc}", name=f"qTb{hc}")
               for hc in range(HC)]

        # ---- preamble: transposes + projections (scoped PSUM pool) ----
        with tc.tile_pool(name="psum_tr", bufs=2, space="PSUM") as trp:
            # query^T: [s_p, s_c, q]
            qT_bf = sb.tile([128, SC, QL], BF16)
            for sc in range(SC):
                pt = trp.tile([128, 128], BF16, tag="tr")
                nc.tensor.transpose(pt, q_bf[:, sc * 128:(sc + 1) * 128], ident)
                nc.vector.tensor_copy(out=qT_bf[:, sc, :], in_=pt)
            # memory^T: [s_p, s_c, m]  (mt-outer so the kT projection's
            # first m-half can start after 4 m-tiles are transposed)
            memT_bf = sb.tile([128, SC, ML], BF16)
            for mt in range(MT):
                for sc in range(SC):
                    pt = trp.tile([128, 128], BF16, tag="tr")
                    nc.tensor.transpose(
                        pt, mem_bf[:, mt, sc * 128:(sc + 1) * 128], ident
                    )
                    nc.vector.tensor_copy(
                        out=memT_bf[:, sc, mt * 128:(mt + 1) * 128], in_=pt
                    )

            # qT projection: accumulate over s-chunks
            for hc in range(HC):
                pq = trp.tile([128, QL], FP32, tag="pq")
                for sc in range(SC):
                    nc.tensor.matmul(
                        pq,
                        wqt_bf[:, sc, hc * 128:(hc + 1) * 128],
                        qT_bf[:, sc, :],
                        start=(sc == 0),
                        stop=(sc == SC - 1),
                    )
                # qTb = pq + (bq+bk), per-partition scalar add
                nc.vector.tensor_scalar(
                    out=qTb[hc], in0=pq,
                    scalar1=bqk_sb[:, hc:hc + 1], scalar2=None,
                    op0=mybir.AluOpType.add,
                )

            # kT projection straight into resident PSUM
            for mh in range(2):
                for hc in range(HC):
                    out_sl = kt_ps[hc][:, mh * 512:(mh + 1) * 512]
                    for sc in range(SC):
                        nc.tensor.matmul(
                            out_sl,
                            wkt_bf[:, sc, hc * 128:(hc + 1) * 128],
                            memT_bf[:, sc, mh * 512:(mh + 1) * 512],
                            start=(sc == 0),
                            stop=(sc == SC - 1),
                        )

        # ---- v2: DVE tanh offload setup ----
        # For a subset of (q, hc=1) jobs, DVE computes r = 1/(e^{2q}e^{2k}+1)
        # and the PE dot uses weights -2*wl on r; tanh = 1-2r up to the
        # per-row constant sum(wl) which softmax cancels.
        dve_q = set()
        if N_DVE_JOBS > 0:
            step = QL / N_DVE_JOBS
            dve_q = {int(i * step) for i in range(N_DVE_JOBS)}
        gps_q = set()
        if N_GPS_U > 0 and dve_q:
            dl = sorted(dve_q)
            stepg = len(dl) / N_GPS_U
            gps_q = {dl[int(i * stepg)] for i in range(N_GPS_U)}
        if dve_q:
            A1 = sb.tile([128, QL], FP32)        # e^{2*qTb[hc=1]}
            nc.scalar.activation(out=A1, in_=qTb[1],
                                 func=mybir.ActivationFunctionType.Exp,
                                 scale=2.0)
            Bx1 = sb.tile([128, ML], FP32)       # e^{2*kT[hc=1]}
            nc.scalar.activation(out=Bx1, in_=kt_ps[1],
                                 func=mybir.ActivationFunctionType.Exp,
                                 scale=2.0)
            wlneg_sb = const.tile([128, 32], BF16)  # -2*wl hc=1, replicated
            nc.gpsimd.dma_start(out=wlneg_sb, in_=wlneg[:])
            upool = ctx.enter_context(tc.tile_pool(name="u", bufs=3))
            rpool = ctx.enter_context(tc.tile_pool(name="r", bufs=6))

        # ---- main loop ----
        # Matmul PSUM outputs may only start at partitions {0,32,64,96}
        # (PE col-groups), so queries go 4-per-PSUM-region at those rows,
        # get evacuated to SBUF staging, and a small SBUF->SBUF DMA gathers
        # the 4 rows onto consecutive partitions of the logits tile.
        logits_sb = sb.tile([128, ML], FP32)
        stg_pool = ctx.enter_context(tc.tile_pool(name="stg", bufs=4))

        with tc.tile_pool(name="psum_rg", bufs=2, space="PSUM") as rgp:
            for g in range(QL // 4):
                rg = rgp.tile([128, ML], FP32, tag="rg")
                z0s, h1s = [], []
                for qc in range(4):
                    q = 4 * g + qc
                    z0 = zpool.tile([128, ML], BF16, tag="z0")
                    nc.scalar.activation(
                        out=z0, in_=kt_ps[0],
                        func=mybir.ActivationFunctionType.Tanh,
                        bias=qTb[0][:, q:q + 1], scale=1.0,
                    )
                    z0s.append(z0)
                from concourse.dve_ops import (
                    RECIP_APPROX_FAST_CONSTS as _RC,
                    RECIPROCAL_APPROX_FAST as _RF,
                )
                for qc in range(4):
                    q = 4 * g + qc
                    if q in dve_q:
                        ut = upool.tile([128, ML], FP32, tag="u")
                        ueng = nc.gpsimd if q in gps_q else nc.vector
                        ueng.tensor_scalar(
                            out=ut, in0=Bx1, scalar1=A1[:, q:q + 1],
                            scalar2=1.0, op0=mybir.AluOpType.mult,
                            op1=mybir.AluOpType.add,
                        )
                        # custom-DVE recip, bf16 out (the fp32 restriction
                        # is about the BITWISE_NOT on the *input*; output
                        # goes through the normal dtype converter) - saves
                        # a separate cast pass.
                        rt = rpool.tile([128, ML], BF16, tag="r")
                        nc.vector._custom_dve(
                            _RF, out=rt[:], in0=ut[:],
                            s0=_RC["s0"], s1=_RC["s1"], imm2=_RC["imm2"],
                        )
                        h1s.append((rt, wlneg_sb))
                    else:
                        z1 = rpool.tile([128, ML], BF16, tag="r")
                        nc.scalar.activation(
                            out=z1, in_=kt_ps[1],
                            func=mybir.ActivationFunctionType.Tanh,
                            bias=qTb[1][:, q:q + 1], scale=1.0,
                        )
                        h1s.append((z1, None))
                for qc in range(4):
                    rt, wneg = h1s[qc]
                    for mh in range(2):
                        sl = slice(mh * 512, (mh + 1) * 512)
                        nc.tensor.matmul(
                            rg[32 * qc:32 * qc + 32, sl],
                            wl_bf[:, 0, :],
                            z0s[qc][:, sl],
                            start=True, stop=False,
                            tile_position=(0, 32 * qc),
                        )
                        nc.tensor.matmul(
                            rg[32 * qc:32 * qc + 32, sl],
                            wneg[:] if wneg is not None else wl_bf[:, 1, :],
                            rt[:, sl],
                            start=False, stop=True,
                            tile_position=(0, 32 * qc),
                        )
                stage = stg_pool.tile([128, ML], FP32, tag="stage")
                st_ap = stage[:]
                if N_ACT_EVAC and g % max(1, 32 // N_ACT_EVAC) == 0:
                    nc.scalar.copy(out=stage, in_=rg)
                else:
                    nc.vector.tensor_copy(out=stage, in_=rg)
                p_stride = st_ap.ap[0][0]
                nc.sync.dma_start(
                    out=logits_sb[4 * g:4 * g + 4, :],
                    in_=bass.AP(tensor=st_ap.tensor, offset=st_ap.offset,
                                ap=[[32 * p_stride, 4]] + list(st_ap.ap)[1:]),
                )

        psum_at = ctx.enter_context(tc.tile_pool(name="psum_at", bufs=1, space="PSUM"))

        # ---- masked softmax over m (rows = q on partitions) ----
        # |logits| <= H*max|wl| = 16, so exp() is safe without the rowmax
        # shift; softmax is shift-invariant so the result is identical.
        # The DVE mask-add also stands between the 32 gather DMAs and the
        # ACT exp: an ACT instruction waiting directly on that many DMA
        # queues hangs the exec unit (observed NRT_EXEC_UNIT_UNRECOVERABLE).
        lmask = sb.tile([128, ML], FP32)
        nc.vector.tensor_add(lmask, logits_sb, maskb)
        ewm = sb.tile([128, ML], FP32)
        rsum = sb.tile([128, 1], FP32)
        nc.scalar.activation(
            out=ewm, in_=lmask, func=mybir.ActivationFunctionType.Exp,
            scale=1.0,
        )
        nc.vector.tensor_reduce(
            out=rsum, in_=ewm, axis=mybir.AxisListType.X,
            op=mybir.AluOpType.add,
        )
        rinv = sb.tile([128, 1], FP32)
        nc.vector.reciprocal(out=rinv, in_=rsum)

        wout = sb.tile([128, ML], FP32)
        nc.gpsimd.tensor_scalar(
            out=wout, in0=ewm, scalar1=rinv, scalar2=None,
            op0=mybir.AluOpType.mult,
        )
        nc.sync.dma_start(out=weights_o[:], in_=wout)

        # ---- attns = (ewm @ memory) * rinv ----
        ewm_bf = sb.tile([128, ML], BF16)
        nc.gpsimd.tensor_copy(out=ewm_bf, in_=ewm)
        wT_sb = sb.tile([128, MT, 128], BF16)         # ewm^T: [m_p, m_t, q]
        for mt in range(MT):
            pt = psum_at.tile([128, 128], BF16, tag="wt")
            nc.tensor.transpose(pt, ewm_bf[:, mt * 128:(mt + 1) * 128], ident)
            nc.vector.tensor_copy(out=wT_sb[:, mt, :], in_=pt)
        pa = psum_at.tile([128, QS], FP32, tag="pa")
        for mt in range(MT):
            nc.tensor.matmul(
                pa, wT_sb[:, mt, :], mem_bf[:, mt, :],
                start=(mt == 0), stop=(mt == MT - 1),
            )
        attns_sb = sb.tile([128, QS], FP32)
        nc.vector.tensor_scalar(
            out=attns_sb, in0=pa, scalar1=rinv, scalar2=None,
            op0=mybir.AluOpType.mult,
        )
        nc.sync.dma_start(out=attns_o[:], in_=attns_sb)

    nc.compile()
    return nc


_NC_CACHE = None


def _get_nc():
    global _NC_CACHE
    if _NC_CACHE is None:
        _NC_CACHE = build_kernel()
    return _NC_CACHE


def make_in_maps(inputs):
    wqt = np.ascontiguousarray(np.asarray(inputs["Wq"], np.float32).T)
    wkt = np.ascontiguousarray(np.asarray(inputs["Wk"], np.float32).T)
    bqk_v = (np.asarray(inputs["bq"], np.float32)
             + np.asarray(inputs["bk"], np.float32))
    bqk = np.ascontiguousarray(bqk_v.reshape(HC, 128).T)
    wl_v = np.asarray(inputs["wl"], np.float32)
    wlrep = np.ascontiguousarray(
        np.repeat(wl_v.reshape(HC, 128).T[:, :, None], 32, axis=2))
    wlneg = np.ascontiguousarray(
        np.repeat(-2.0 * wl_v[128:256][:, None], 32, axis=1))
    in_maps = []
    for i in range(N_CORES):
        in_maps.append({
            "query": np.ascontiguousarray(inputs["query"][i], np.float32),
            "memory": np.ascontiguousarray(inputs["memory"][i], np.float32),
            "mask": np.ascontiguousarray(inputs["mask"][i]).astype(np.uint8),
            "wqt": wqt,
            "wkt": wkt,
            "bqk": bqk,
            "wlrep": wlrep,
            "wlneg": wlneg,
        })
    return in_maps


def kernel(**inputs):
    from concourse.bass_utils import run_bass_kernel_spmd

    nc = _get_nc()
    in_maps = make_in_maps(inputs)
    res = run_bass_kernel_spmd(nc, in_maps, list(range(N_CORES)))
    attns = np.stack([res.results[i]["attns"] for i in range(N_CORES)])
    weights = np.stack([res.results[i]["weights"] for i in range(N_CORES)])
    return attns, weights


# revision 27
# speedup vs baseline: 1.0549x; 1.0374x over previous
"""Bahdanau additive attention on 8 Trainium2 NeuronCores.

Per-core program (data-parallel over batch B=8, one batch element per core):
  qT[h,q]   = Wq @ query.T + (bq+bk)       h on partitions (2 chunks of 128)
  kT[h,m]   = Wk @ memory.T                h on partitions, PSUM-resident
  z[h,m]    = tanh(kT + qT[:,q])           ACT, bias = per-partition qT column
  logits[q,m] = sum_h wl[h] * z[h,m]       PE, M=32-replicated matmuls into
                                           PSUM col-group rows {0,32,64,96}
  weights   = softmax_m(logits + mask*-1e18)
  attns     = weights @ memory

Work split across engines (per core, per q-row): ACT computes tanh for
hc=0 and for 128-N_DVE_JOBS of the hc=1 jobs; for the rest, GPSIMD computes
u = e^{2qT}*e^{2kT}+1 and the DVE a fast approximate reciprocal r=1/u
(tanh = 1-2r up to a per-row constant that softmax cancels), so all three
elementwise engines run the hot loop concurrently. The additive bias `bl`
shifts every logit uniformly -> softmax-invariant -> dropped. The mask is
applied as a -1e18 additive fill before exp, exactly like the reference.
"""

import os
from contextlib import ExitStack

import numpy as np

import concourse.bass as bass
import concourse.bacc as bacc
import concourse.tile as tile
from concourse import mybir
from concourse.masks import make_identity

B, QL, ML = 8, 128, 1024
QS, KS, H = 512, 512, 256
N_CORES = 8

FP32 = mybir.dt.float32
BF16 = mybir.dt.bfloat16
U8 = mybir.dt.uint8

SC = QS // 128  # 4 s-chunks
HC = H // 128   # 2 h-chunks
MT = ML // 128  # 8 m-tiles

# v2: offload this many of the 128 (q, hc=1) tanh jobs from ACT to DVE via
# tanh(x) = 1 - 2/(e^{2q}e^{2k}+1)  (the per-row constant sum(wl) cancels in
# softmax). 0 disables the DVE path entirely.
N_DVE_JOBS = int(os.environ.get("BAHDANAU_DVE_JOBS", "114"))
# of those, how many u=A*B+1 passes run on GPSIMD instead of DVE
N_GPS_U = int(os.environ.get("BAHDANAU_GPS_U", "114"))
# of the 32 region evacuations, how many run on ACT instead of DVE
N_ACT_EVAC = int(os.environ.get("BAHDANAU_ACT_EVAC", "8"))
FP32R = mybir.dt.float32r


def build_kernel():
    nc = bacc.Bacc(None, target_bir_lowering=False)

    # ---- DRAM parameters (per-core slices supplied host-side) ----
    query = nc.declare_dram_parameter("query", [QL, QS], FP32, isOutput=False)
    memory = nc.declare_dram_parameter("memory", [ML, KS], FP32, isOutput=False)
    mask = nc.declare_dram_parameter("mask", [ML], U8, isOutput=False)
    wqt = nc.declare_dram_parameter("wqt", [QS, H], FP32, isOutput=False)
    wkt = nc.declare_dram_parameter("wkt", [KS, H], FP32, isOutput=False)
    bqk = nc.declare_dram_parameter("bqk", [128, HC], FP32, isOutput=False)
    wlrep = nc.declare_dram_parameter("wlrep", [128, HC, 32], FP32, isOutput=False)
    wlneg = nc.declare_dram_parameter("wlneg", [128, 32], FP32, isOutput=False)
    attns_o = nc.declare_dram_parameter("attns", [QL, QS], FP32, isOutput=True)
    weights_o = nc.declare_dram_parameter("weights", [QL, ML], FP32, isOutput=True)

    with ExitStack() as ctx:
        tc = ctx.enter_context(tile.TileContext(nc))
        const = ctx.enter_context(tc.tile_pool(name="const", bufs=1))
        sb = ctx.enter_context(tc.tile_pool(name="sb", bufs=1))
        zpool = ctx.enter_context(tc.tile_pool(name="z", bufs=6))
        # PSUM stack: kT (4 banks) lives for the whole kernel.
        psum_kt = ctx.enter_context(tc.tile_pool(name="psum_kt", bufs=1, space="PSUM"))

        # ---- constant / input loads ----
        ident = const.tile([128, 128], BF16)
        make_identity(nc, ident)

        # bf16 casting loads (gpsimd SWDGE does dtype conversion)
        q_bf = sb.tile([128, QS], BF16)               # [q, s]
        nc.gpsimd.dma_start(out=q_bf, in_=query[:])
        mem_bf = sb.tile([128, MT, KS], BF16)         # [m_p, m_t, s]
        mem_r = memory[:].rearrange("(t p) s -> p t s", p=128)
        for mt in range(MT):
            nc.gpsimd.dma_start(out=mem_bf[:, mt, :], in_=mem_r[:, mt, :])
        wqt_bf = sb.tile([128, SC, H], BF16)          # [s_p, s_c, h]
        nc.gpsimd.dma_start(
            out=wqt_bf, in_=wqt[:].rearrange("(c p) h -> p c h", p=128)
        )
        wkt_bf = sb.tile([128, SC, H], BF16)
        nc.gpsimd.dma_start(
            out=wkt_bf, in_=wkt[:].rearrange("(c p) h -> p c h", p=128)
        )
        bqk_sb = const.tile([128, HC], FP32)          # [h_p, h_c]
        nc.gpsimd.dma_start(out=bqk_sb, in_=bqk[:])
        # wl replicated 32x along free dim (host-side layout): lhsT [128, 32]
        # per h-chunk, so M=32 matmuls fill a whole PE col-group (same cost
        # as M=1).
        wl_bf = const.tile([128, HC, 32], BF16)       # [h_p, h_c, rep]
        nc.gpsimd.dma_start(out=wl_bf, in_=wlrep[:])

        mask_u8 = sb.tile([128, ML], U8)
        m_ap = mask[:]
        nc.gpsimd.dma_start(
            out=mask_u8,
            in_=bass.AP(tensor=m_ap.tensor, offset=m_ap.offset,
                        ap=[[0, 128]] + list(m_ap.ap)),
        )
        maskb = sb.tile([128, ML], FP32)              # -1e18 at masked, 0 else
        nc.vector.tensor_scalar(
            out=maskb, in0=mask_u8, scalar1=-1e18, scalar2=None,
            op0=mybir.AluOpType.mult,
        )

        # kT PSUM-resident: 2 x [h_p, m] fp32 = 4 banks (separate tiles so
        # hc=0 consumers don't falsely depend on hc=1 writers)
        kt_ps = [psum_kt.tile([128, ML], FP32, tag=f"kt{hc}", name=f"kt{hc}")
                 for hc in range(HC)]

        qTb = [sb.tile([128, QL], FP32, tag=f"qTb{hc}", name=f"qTb{hc}")
               for hc in range(HC)]

        # ---- preamble: transposes + projections (scoped PSUM pool) ----
        with tc.tile_pool(name="psum_tr", bufs=2, space="PSUM") as trp:
            # query^T: [s_p, s_c, q]
            qT_bf = sb.tile([128, SC, QL], BF16)
            for sc in range(SC):
                pt = trp.tile([128, 128], BF16, tag="tr")
                nc.tensor.transpose(pt, q_bf[:, sc * 128:(sc + 1) * 128], ident)
                nc.vector.tensor_copy(out=qT_bf[:, sc, :], in_=pt)
            # memory^T: [s_p, s_c, m]  (mt-outer so the kT projection's
            # first m-half can start after 4 m-tiles are transposed)
            memT_bf = sb.tile([128, SC, ML], BF16)
            for mt in range(MT):
                for sc in range(SC):
                    pt = trp.tile([128, 128], BF16, tag="tr")
                    nc.tensor.transpose(
                        pt, mem_bf[:, mt, sc * 128:(sc + 1) * 128], ident
                    )
                    nc.vector.tensor_copy(
                        out=memT_bf[:, sc, mt * 128:(mt + 1) * 128], in_=pt
                    )

            # qT projection: accumulate over s-chunks
            for hc in range(HC):
                pq = trp.tile([128, QL], FP32, tag="pq")
                for sc in range(SC):
                    nc.tensor.matmul(
                        pq,
                        wqt_bf[:, sc, hc * 128:(hc + 1) * 128],
                        qT_bf[:, sc, :],
                        start=(sc == 0),
                        stop=(sc == SC - 1),
                    )
                # qTb = pq + (bq+bk), per-partition scalar add
                nc.vector.tensor_scalar(
                    out=qTb[hc], in0=pq,
                    scalar1=bqk_sb[:, hc:hc + 1], scalar2=None,
                    op0=mybir.AluOpType.add,
                )

            # kT projection straight into resident PSUM
            for mh in range(2):
                for hc in range(HC):
                    out_sl = kt_ps[hc][:, mh * 512:(mh + 1) * 512]
                    for sc in range(SC):
                        nc.tensor.matmul(
                            out_sl,
                            wkt_bf[:, sc, hc * 128:(hc + 1) * 128],
                            memT_bf[:, sc, mh * 512:(mh + 1) * 512],
                            start=(sc == 0),
                            stop=(sc == SC - 1),
                        )

        # ---- v2: DVE tanh offload setup ----
        # For a subset of (q, hc=1) jobs, DVE computes r = 1/(e^{2q}e^{2k}+1)
        # and the PE dot uses weights -2*wl on r; tanh = 1-2r up to the
        # per-row constant sum(wl) which softmax cancels.
        dve_q = set()
        if N_DVE_JOBS > 0:
            step = QL / N_DVE_JOBS
            dve_q = {int(i * step) for i in range(N_DVE_JOBS)}
        gps_q = set()
        if N_GPS_U > 0 and dve_q:
            dl = sorted(dve_q)
            stepg = len(dl) / N_GPS_U
            gps_q = {dl[int(i * stepg)] for i in range(N_GPS_U)}
        if dve_q:
            A1 = sb.tile([128, QL], FP32)        # e^{2*qTb[hc=1]}
            nc.scalar.activation(out=A1, in_=qTb[1],
                                 func=mybir.ActivationFunctionType.Exp,
                                 scale=2.0)
            Bx1 = sb.tile([128, ML], FP32)       # e^{2*kT[hc=1]}
            nc.scalar.activation(out=Bx1, in_=kt_ps[1],
                                 func=mybir.ActivationFunctionType.Exp,
                                 scale=2.0)
            wlneg_sb = const.tile([128, 32], BF16)  # -2*wl hc=1, replicated
            nc.gpsimd.dma_start(out=wlneg_sb, in_=wlneg[:])
            upool = ctx.enter_context(tc.tile_pool(name="u", bufs=3))
            rpool = ctx.enter_context(tc.tile_pool(name="r", bufs=6))

        # ---- main loop ----
        # Matmul PSUM outputs may only start at partitions {0,32,64,96}
        # (PE col-groups), so queries go 4-per-PSUM-region at those rows,
        # get evacuated to SBUF staging, and a small SBUF->SBUF DMA gathers
        # the 4 rows onto consecutive partitions of the logits tile.
        logits_sb = sb.tile([128, ML], FP32)
        stg_pool = ctx.enter_context(tc.tile_pool(name="stg", bufs=4))

        with tc.tile_pool(name="psum_rg", bufs=2, space="PSUM") as rgp:
            for g in range(QL // 4):
                rg = rgp.tile([128, ML], FP32, tag="rg")
                z0s, h1s = [], []
                for qc in range(4):
                    q = 4 * g + qc
                    z0 = zpool.tile([128, ML], BF16, tag="z0")
                    nc.scalar.activation(
                        out=z0, in_=kt_ps[0],
                        func=mybir.ActivationFunctionType.Tanh,
                        bias=qTb[0][:, q:q + 1], scale=1.0,
                    )
                    z0s.append(z0)
                from concourse.dve_ops import (
                    RECIP_APPROX_FAST_CONSTS as _RC,
                    RECIPROCAL_APPROX_FAST as _RF,
                )
                for qc in range(4):
                    q = 4 * g + qc
                    if q in dve_q:
                        ut = upool.tile([128, ML], FP32, tag="u")
                        ueng = nc.gpsimd if q in gps_q else nc.vector
                        ueng.tensor_scalar(
                            out=ut, in0=Bx1, scalar1=A1[:, q:q + 1],
                            scalar2=1.0, op0=mybir.AluOpType.mult,
                            op1=mybir.AluOpType.add,
                        )
                        # custom-DVE recip, bf16 out (the fp32 restriction
                        # is about the BITWISE_NOT on the *input*; output
                        # goes through the normal dtype converter) - saves
                        # a separate cast pass.
                        rt = rpool.tile([128, ML], BF16, tag="r")
                        nc.vector._custom_dve(
                            _RF, out=rt[:], in0=ut[:],
                            s0=_RC["s0"], s1=_RC["s1"], imm2=_RC["imm2"],
                        )
                        h1s.append((rt, wlneg_sb))
                    else:
                        z1 = rpool.tile([128, ML], BF16, tag="r")
                        nc.scalar.activation(
                            out=z1, in_=kt_ps[1],
                            func=mybir.ActivationFunctionType.Tanh,
                            bias=qTb[1][:, q:q + 1], scale=1.0,
                        )
                        h1s.append((z1, None))
                for qc in range(4):
                    rt, wneg = h1s[qc]
                    for mh in range(2):
                        sl = slice(mh * 512, (mh + 1) * 512)
                        nc.tensor.matmul(
                            rg[32 * qc:32 * qc + 32, sl],
                            wl_bf[:, 0, :],
                            z0s[qc][:, sl],
                            start=True, stop=False,
                            tile_position=(0, 32 * qc),
                        )
                        nc.tensor.matmul(
                            rg[32 * qc:32 * qc + 32, sl],
                            wneg[:] if wneg is not None else wl_bf[:, 1, :],
                            rt[:, sl],
                            start=False, stop=True,
                            tile_position=(0, 32 * qc),
                        )
                stage = stg_pool.tile([128, ML], FP32, tag="stage")
                st_ap = stage[:]
                if N_ACT_EVAC and g % max(1, 32 // N_ACT_EVAC) == 0:
                    nc.scalar.copy(out=stage, in_=rg)
                else:
                    nc.vector.tensor_copy(out=stage, in_=rg)
                p_stride = st_ap.ap[0][0]
                nc.sync.dma_start(
                    out=logits_sb[4 * g:4 * g + 4, :],
                    in_=bass.AP(tensor=st_ap.tensor, offset=st_ap.offset,
                                ap=[[32 * p_stride, 4]] + list(st_ap.ap)[1:]),
                )

        psum_at = ctx.enter_context(tc.tile_pool(name="psum_at", bufs=1, space="PSUM"))

        # ---- masked softmax over m (rows = q on partitions) ----
        # |logits| <= H*max|wl| = 16, so exp() is safe without the rowmax
        # shift; softmax is shift-invariant so the result is identical.
        # The DVE mask-add also stands between the 32 gather DMAs and the
        # ACT exp: an ACT instruction waiting directly on that many DMA
        # queues hangs the exec unit (observed NRT_EXEC_UNIT_UNRECOVERABLE).
        lmask = sb.tile([128, ML], FP32)
        nc.vector.tensor_add(lmask, logits_sb, maskb)
        ewm = sb.tile([128, ML], FP32)
        rsum = sb.tile([128, 1], FP32)
        nc.scalar.activation(
            out=ewm, in_=lmask, func=mybir.ActivationFunctionType.Exp,
            scale=1.0,
        )
        nc.vector.tensor_reduce(
            out=rsum, in_=ewm, axis=mybir.AxisListType.X,
            op=mybir.AluOpType.add,
        )
        rinv = sb.tile([128, 1], FP32)
        nc.vector.reciprocal(out=rinv, in_=rsum)

        wout = sb.tile([128, ML], FP32)
        nc.vector.tensor_scalar(
            out=wout, in0=ewm, scalar1=rinv, scalar2=None,
            op0=mybir.AluOpType.mult,
        )
        nc.sync.dma_start(out=weights_o[:], in_=wout)

        # ---- attns = (ewm @ memory) * rinv ----
        ewm_bf = sb.tile([128, ML], BF16)
        nc.vector.tensor_copy(out=ewm_bf, in_=ewm)
        wT_sb = sb.tile([128, MT, 128], BF16)         # ewm^T: [m_p, m_t, q]
        for mt in range(MT):
            pt = psum_at.tile([128, 128], BF16, tag="wt")
            nc.tensor.transpose(pt, ewm_bf[:, mt * 128:(mt + 1) * 128], ident)
            nc.vector.tensor_copy(out=wT_sb[:, mt, :], in_=pt)
        pa = psum_at.tile([128, QS], FP32, tag="pa")
        for mt in range(MT):
            nc.tensor.matmul(
                pa, wT_sb[:, mt, :], mem_bf[:, mt, :],
                start=(mt == 0), stop=(mt == MT - 1),
            )
        attns_sb = sb.tile([128, QS], FP32)
        nc.vector.tensor_scalar(
            out=attns_sb, in0=pa, scalar1=rinv, scalar2=None,
            op0=mybir.AluOpType.mult,
        )
        nc.sync.dma_start(out=attns_o[:], in_=attns_sb)

    nc.compile()
    return nc


_NC_CACHE = None


def _get_nc():
    global _NC_CACHE
    if _NC_CACHE is None:
        _NC_CACHE = build_kernel()
    return _NC_CACHE


def make_in_maps(inputs):
    wqt = np.ascontiguousarray(np.asarray(inputs["Wq"], np.float32).T)
    wkt = np.ascontiguousarray(np.asarray(inputs["Wk"], np.float32).T)
    bqk_v = (np.asarray(inputs["bq"], np.float32)
             + np.asarray(inputs["bk"], np.float32))
    bqk = np.ascontiguousarray(bqk_v.reshape(HC, 128).T)
    wl_v = np.asarray(inputs["wl"], np.float32)
    wlrep = np.ascontiguousarray(
        np.repeat(wl_v.reshape(HC, 128).T[:, :, None], 32, axis=2))
    wlneg = np.ascontiguousarray(
        np.repeat(-2.0 * wl_v[128:256][:, None], 32, axis=1))
    in_maps = []
    for i in range(N_CORES):
        in_maps.append({
            "query": np.ascontiguousarray(inputs["query"][i], np.float32),
            "memory": np.ascontiguousarray(inputs["memory"][i], np.float32),
            "mask": np.ascontiguousarray(inputs["mask"][i]).astype(np.uint8),
            "wqt": wqt,
            "wkt": wkt,
            "bqk": bqk,
            "wlrep": wlrep,
            "wlneg": wlneg,
        })
    return in_maps


def kernel(**inputs):
    from concourse.bass_utils import run_bass_kernel_spmd

    nc = _get_nc()
    in_maps = make_in_maps(inputs)
    res = run_bass_kernel_spmd(nc, in_maps, list(range(N_CORES)))
    attns = np.stack([res.results[i]["attns"] for i in range(N_CORES)])
    weights = np.stack([res.results[i]["weights"] for i in range(N_CORES)])
    return attns, weights


# revision 28
# speedup vs baseline: 1.0575x; 1.0025x over previous
"""Bahdanau additive attention on 8 Trainium2 NeuronCores.

Per-core program (data-parallel over batch B=8, one batch element per core):
  qT[h,q]   = Wq @ query.T + (bq+bk)       h on partitions (2 chunks of 128)
  kT[h,m]   = Wk @ memory.T                h on partitions, PSUM-resident
  z[h,m]    = tanh(kT + qT[:,q])           ACT, bias = per-partition qT column
  logits[q,m] = sum_h wl[h] * z[h,m]       PE, M=32-replicated matmuls into
                                           PSUM col-group rows {0,32,64,96}
  weights   = softmax_m(logits + mask*-1e18)
  attns     = weights @ memory

Work split across engines (per core, per q-row): ACT computes tanh for
hc=0 and for 128-N_DVE_JOBS of the hc=1 jobs; for the rest, GPSIMD computes
u = e^{2qT}*e^{2kT}+1 and the DVE a fast approximate reciprocal r=1/u
(tanh = 1-2r up to a per-row constant that softmax cancels), so all three
elementwise engines run the hot loop concurrently. The additive bias `bl`
shifts every logit uniformly -> softmax-invariant -> dropped. The mask is
applied as a -1e18 additive fill before exp, exactly like the reference.
"""

import os
from contextlib import ExitStack

import numpy as np

import concourse.bass as bass
import concourse.bacc as bacc
import concourse.tile as tile
from concourse import mybir
from concourse.masks import make_identity

B, QL, ML = 8, 128, 1024
QS, KS, H = 512, 512, 256
N_CORES = 8

FP32 = mybir.dt.float32
BF16 = mybir.dt.bfloat16
U8 = mybir.dt.uint8

SC = QS // 128  # 4 s-chunks
HC = H // 128   # 2 h-chunks
MT = ML // 128  # 8 m-tiles

# v2: offload this many of the 128 (q, hc=1) tanh jobs from ACT to DVE via
# tanh(x) = 1 - 2/(e^{2q}e^{2k}+1)  (the per-row constant sum(wl) cancels in
# softmax). 0 disables the DVE path entirely.
N_DVE_JOBS = int(os.environ.get("BAHDANAU_DVE_JOBS", "114"))
# of those, how many u=A*B+1 passes run on GPSIMD instead of DVE
N_GPS_U = int(os.environ.get("BAHDANAU_GPS_U", "114"))
# of the 32 region evacuations, how many run on ACT instead of DVE
N_ACT_EVAC = int(os.environ.get("BAHDANAU_ACT_EVAC", "8"))
FP32R = mybir.dt.float32r


def build_kernel():
    nc = bacc.Bacc(None, target_bir_lowering=False)

    # ---- DRAM parameters (per-core slices supplied host-side) ----
    query = nc.declare_dram_parameter("query", [QL, QS], FP32, isOutput=False)
    memory = nc.declare_dram_parameter("memory", [ML, KS], FP32, isOutput=False)
    mask = nc.declare_dram_parameter("mask", [ML], U8, isOutput=False)
    wqt = nc.declare_dram_parameter("wqt", [QS, H], FP32, isOutput=False)
    wkt = nc.declare_dram_parameter("wkt", [KS, H], FP32, isOutput=False)
    bqk = nc.declare_dram_parameter("bqk", [128, HC], FP32, isOutput=False)
    wlrep = nc.declare_dram_parameter("wlrep", [128, HC, 32], FP32, isOutput=False)
    wlneg = nc.declare_dram_parameter("wlneg", [128, 32], FP32, isOutput=False)
    attns_o = nc.declare_dram_parameter("attns", [QL, QS], FP32, isOutput=True)
    weights_o = nc.declare_dram_parameter("weights", [QL, ML], FP32, isOutput=True)

    with ExitStack() as ctx:
        tc = ctx.enter_context(tile.TileContext(nc))
        const = ctx.enter_context(tc.tile_pool(name="const", bufs=1))
        sb = ctx.enter_context(tc.tile_pool(name="sb", bufs=1))
        zpool = ctx.enter_context(tc.tile_pool(name="z", bufs=6))
        # PSUM stack: kT (4 banks) lives for the whole kernel.
        psum_kt = ctx.enter_context(tc.tile_pool(name="psum_kt", bufs=1, space="PSUM"))

        # ---- constant / input loads ----
        ident = const.tile([128, 128], BF16)
        make_identity(nc, ident)

        # bf16 casting loads (gpsimd SWDGE does dtype conversion)
        q_bf = sb.tile([128, QS], BF16)               # [q, s]
        nc.gpsimd.dma_start(out=q_bf, in_=query[:])
        mem_bf = sb.tile([128, MT, KS], BF16)         # [m_p, m_t, s]
        mem_r = memory[:].rearrange("(t p) s -> p t s", p=128)
        for mt in range(MT):
            nc.gpsimd.dma_start(out=mem_bf[:, mt, :], in_=mem_r[:, mt, :])
        wqt_bf = sb.tile([128, SC, H], BF16)          # [s_p, s_c, h]
        nc.gpsimd.dma_start(
            out=wqt_bf, in_=wqt[:].rearrange("(c p) h -> p c h", p=128)
        )
        wkt_bf = sb.tile([128, SC, H], BF16)
        nc.gpsimd.dma_start(
            out=wkt_bf, in_=wkt[:].rearrange("(c p) h -> p c h", p=128)
        )
        bqk_sb = const.tile([128, HC], FP32)          # [h_p, h_c]
        nc.gpsimd.dma_start(out=bqk_sb, in_=bqk[:])
        # wl replicated 32x along free dim (host-side layout): lhsT [128, 32]
        # per h-chunk, so M=32 matmuls fill a whole PE col-group (same cost
        # as M=1).
        wl_bf = const.tile([128, HC, 32], BF16)       # [h_p, h_c, rep]
        nc.gpsimd.dma_start(out=wl_bf, in_=wlrep[:])

        mask_u8 = sb.tile([128, ML], U8)
        m_ap = mask[:]
        nc.gpsimd.dma_start(
            out=mask_u8,
            in_=bass.AP(tensor=m_ap.tensor, offset=m_ap.offset,
                        ap=[[0, 128]] + list(m_ap.ap)),
        )
        maskb = sb.tile([128, ML], FP32)              # -1e18 at masked, 0 else
        nc.vector.tensor_scalar(
            out=maskb, in0=mask_u8, scalar1=-1e18, scalar2=None,
            op0=mybir.AluOpType.mult,
        )

        # kT PSUM-resident: 2 x [h_p, m] fp32 = 4 banks (separate tiles so
        # hc=0 consumers don't falsely depend on hc=1 writers)
        kt_ps = [psum_kt.tile([128, ML], FP32, tag=f"kt{hc}", name=f"kt{hc}")
                 for hc in range(HC)]

        qTb = [sb.tile([128, QL], FP32, tag=f"qTb{hc}", name=f"qTb{hc}")
               for hc in range(HC)]

        # ---- preamble: transposes + projections (scoped PSUM pool) ----
        with tc.tile_pool(name="psum_tr", bufs=2, space="PSUM") as trp:
            # query^T: [s_p, s_c, q]
            qT_bf = sb.tile([128, SC, QL], BF16)
            for sc in range(SC):
                pt = trp.tile([128, 128], BF16, tag="tr")
                nc.tensor.transpose(pt, q_bf[:, sc * 128:(sc + 1) * 128], ident)
                nc.vector.tensor_copy(out=qT_bf[:, sc, :], in_=pt)
            # memory^T: [s_p, s_c, m]  (mt-outer so the kT projection's
            # first m-half can start after 4 m-tiles are transposed)
            memT_bf = sb.tile([128, SC, ML], BF16)
            for mt in range(MT):
                for sc in range(SC):
                    pt = trp.tile([128, 128], BF16, tag="tr")
                    nc.tensor.transpose(
                        pt, mem_bf[:, mt, sc * 128:(sc + 1) * 128], ident
                    )
                    nc.vector.tensor_copy(
                        out=memT_bf[:, sc, mt * 128:(mt + 1) * 128], in_=pt
                    )

            # qT projection: accumulate over s-chunks
            for hc in range(HC):
                pq = trp.tile([128, QL], FP32, tag="pq")
                for sc in range(SC):
                    nc.tensor.matmul(
                        pq,
                        wqt_bf[:, sc, hc * 128:(hc + 1) * 128],
                        qT_bf[:, sc, :],
                        start=(sc == 0),
                        stop=(sc == SC - 1),
                    )
                # qTb = pq + (bq+bk), per-partition scalar add
                nc.vector.tensor_scalar(
                    out=qTb[hc], in0=pq,
                    scalar1=bqk_sb[:, hc:hc + 1], scalar2=None,
                    op0=mybir.AluOpType.add,
                )

            # kT projection straight into resident PSUM
            for mh in range(2):
                for hc in range(HC):
                    out_sl = kt_ps[hc][:, mh * 512:(mh + 1) * 512]
                    for sc in range(SC):
                        nc.tensor.matmul(
                            out_sl,
                            wkt_bf[:, sc, hc * 128:(hc + 1) * 128],
                            memT_bf[:, sc, mh * 512:(mh + 1) * 512],
                            start=(sc == 0),
                            stop=(sc == SC - 1),
                        )

        # ---- v2: DVE tanh offload setup ----
        # For a subset of (q, hc=1) jobs, DVE computes r = 1/(e^{2q}e^{2k}+1)
        # and the PE dot uses weights -2*wl on r; tanh = 1-2r up to the
        # per-row constant sum(wl) which softmax cancels.
        dve_q = set()
        if N_DVE_JOBS > 0:
            step = QL / N_DVE_JOBS
            dve_q = {int(i * step) for i in range(N_DVE_JOBS)}
        gps_q = set()
        if N_GPS_U > 0 and dve_q:
            dl = sorted(dve_q)
            stepg = len(dl) / N_GPS_U
            gps_q = {dl[int(i * stepg)] for i in range(N_GPS_U)}
        if dve_q:
            A1 = sb.tile([128, QL], FP32)        # e^{2*qTb[hc=1]}
            nc.scalar.activation(out=A1, in_=qTb[1],
                                 func=mybir.ActivationFunctionType.Exp,
                                 scale=2.0)
            Bx1 = sb.tile([128, ML], FP32)       # e^{2*kT[hc=1]}
            nc.scalar.activation(out=Bx1, in_=kt_ps[1],
                                 func=mybir.ActivationFunctionType.Exp,
                                 scale=2.0)
            wlneg_sb = const.tile([128, 32], BF16)  # -2*wl hc=1, replicated
            nc.gpsimd.dma_start(out=wlneg_sb, in_=wlneg[:])
            upool = ctx.enter_context(tc.tile_pool(name="u", bufs=4))
            rpool = ctx.enter_context(tc.tile_pool(name="r", bufs=6))

        # ---- main loop ----
        # Matmul PSUM outputs may only start at partitions {0,32,64,96}
        # (PE col-groups), so queries go 4-per-PSUM-region at those rows,
        # get evacuated to SBUF staging, and a small SBUF->SBUF DMA gathers
        # the 4 rows onto consecutive partitions of the logits tile.
        logits_sb = sb.tile([128, ML], FP32)
        stg_pool = ctx.enter_context(tc.tile_pool(name="stg", bufs=6))

        with tc.tile_pool(name="psum_rg", bufs=2, space="PSUM") as rgp:
            for g in range(QL // 4):
                rg = rgp.tile([128, ML], FP32, tag="rg")
                z0s, h1s = [], []
                for qc in range(4):
                    q = 4 * g + qc
                    z0 = zpool.tile([128, ML], BF16, tag="z0")
                    nc.scalar.activation(
                        out=z0, in_=kt_ps[0],
                        func=mybir.ActivationFunctionType.Tanh,
                        bias=qTb[0][:, q:q + 1], scale=1.0,
                    )
                    z0s.append(z0)
                from concourse.dve_ops import (
                    RECIP_APPROX_FAST_CONSTS as _RC,
                    RECIPROCAL_APPROX_FAST as _RF,
                )
                for qc in range(4):
                    q = 4 * g + qc
                    if q in dve_q:
                        ut = upool.tile([128, ML], FP32, tag="u")
                        ueng = nc.gpsimd if q in gps_q else nc.vector
                        ueng.tensor_scalar(
                            out=ut, in0=Bx1, scalar1=A1[:, q:q + 1],
                            scalar2=1.0, op0=mybir.AluOpType.mult,
                            op1=mybir.AluOpType.add,
                        )
                        # custom-DVE recip, bf16 out (the fp32 restriction
                        # is about the BITWISE_NOT on the *input*; output
                        # goes through the normal dtype converter) - saves
                        # a separate cast pass.
                        rt = rpool.tile([128, ML], BF16, tag="r")
                        nc.vector._custom_dve(
                            _RF, out=rt[:], in0=ut[:],
                            s0=_RC["s0"], s1=_RC["s1"], imm2=_RC["imm2"],
                        )
                        h1s.append((rt, wlneg_sb))
                    else:
                        z1 = rpool.tile([128, ML], BF16, tag="r")
                        nc.scalar.activation(
                            out=z1, in_=kt_ps[1],
                            func=mybir.ActivationFunctionType.Tanh,
                            bias=qTb[1][:, q:q + 1], scale=1.0,
                        )
                        h1s.append((z1, None))
                for qc in range(4):
                    rt, wneg = h1s[qc]
                    for mh in range(2):
                        sl = slice(mh * 512, (mh + 1) * 512)
                        nc.tensor.matmul(
                            rg[32 * qc:32 * qc + 32, sl],
                            wl_bf[:, 0, :],
                            z0s[qc][:, sl],
                            start=True, stop=False,
                            tile_position=(0, 32 * qc),
                        )
                        nc.tensor.matmul(
                            rg[32 * qc:32 * qc + 32, sl],
                            wneg[:] if wneg is not None else wl_bf[:, 1, :],
                            rt[:, sl],
                            start=False, stop=True,
                            tile_position=(0, 32 * qc),
                        )
                stage = stg_pool.tile([128, ML], FP32, tag="stage")
                st_ap = stage[:]
                if N_ACT_EVAC and g % max(1, 32 // N_ACT_EVAC) == 0:
                    nc.scalar.copy(out=stage, in_=rg)
                else:
                    nc.vector.tensor_copy(out=stage, in_=rg)
                p_stride = st_ap.ap[0][0]
                nc.sync.dma_start(
                    out=logits_sb[4 * g:4 * g + 4, :],
                    in_=bass.AP(tensor=st_ap.tensor, offset=st_ap.offset,
                                ap=[[32 * p_stride, 4]] + list(st_ap.ap)[1:]),
                )

        psum_at = ctx.enter_context(tc.tile_pool(name="psum_at", bufs=1, space="PSUM"))

        # ---- masked softmax over m (rows = q on partitions) ----
        # |logits| <= H*max|wl| = 16, so exp() is safe without the rowmax
        # shift; softmax is shift-invariant so the result is identical.
        # The DVE mask-add also stands between the 32 gather DMAs and the
        # ACT exp: an ACT instruction waiting directly on that many DMA
        # queues hangs the exec unit (observed NRT_EXEC_UNIT_UNRECOVERABLE).
        lmask = sb.tile([128, ML], FP32)
        nc.vector.tensor_add(lmask, logits_sb, maskb)
        ewm = sb.tile([128, ML], FP32)
        rsum = sb.tile([128, 1], FP32)
        nc.scalar.activation(
            out=ewm, in_=lmask, func=mybir.ActivationFunctionType.Exp,
            scale=1.0,
        )
        nc.vector.tensor_reduce(
            out=rsum, in_=ewm, axis=mybir.AxisListType.X,
            op=mybir.AluOpType.add,
        )
        rinv = sb.tile([128, 1], FP32)
        nc.vector.reciprocal(out=rinv, in_=rsum)

        wout = sb.tile([128, ML], FP32)
        nc.vector.tensor_scalar(
            out=wout, in0=ewm, scalar1=rinv, scalar2=None,
            op0=mybir.AluOpType.mult,
        )
        nc.sync.dma_start(out=weights_o[:], in_=wout)

        # ---- attns = (ewm @ memory) * rinv ----
        ewm_bf = sb.tile([128, ML], BF16)
        nc.vector.tensor_copy(out=ewm_bf, in_=ewm)
        wT_sb = sb.tile([128, MT, 128], BF16)         # ewm^T: [m_p, m_t, q]
        for mt in range(MT):
            pt = psum_at.tile([128, 128], BF16, tag="wt")
            nc.tensor.transpose(pt, ewm_bf[:, mt * 128:(mt + 1) * 128], ident)
            nc.vector.tensor_copy(out=wT_sb[:, mt, :], in_=pt)
        pa = psum_at.tile([128, QS], FP32, tag="pa")
        for mt in range(MT):
            nc.tensor.matmul(
                pa, wT_sb[:, mt, :], mem_bf[:, mt, :],
                start=(mt == 0), stop=(mt == MT - 1),
            )
        attns_sb = sb.tile([128, QS], FP32)
        nc.vector.tensor_scalar(
            out=attns_sb, in0=pa, scalar1=rinv, scalar2=None,
            op0=mybir.AluOpType.mult,
        )
        nc.sync.dma_start(out=attns_o[:], in_=attns_sb)

    nc.compile()
    return nc


_NC_CACHE = None


def _get_nc():
    global _NC_CACHE
    if _NC_CACHE is None:
        _NC_CACHE = build_kernel()
    return _NC_CACHE


def make_in_maps(inputs):
    wqt = np.ascontiguousarray(np.asarray(inputs["Wq"], np.float32).T)
    wkt = np.ascontiguousarray(np.asarray(inputs["Wk"], np.float32).T)
    bqk_v = (np.asarray(inputs["bq"], np.float32)
             + np.asarray(inputs["bk"], np.float32))
    bqk = np.ascontiguousarray(bqk_v.reshape(HC, 128).T)
    wl_v = np.asarray(inputs["wl"], np.float32)
    wlrep = np.ascontiguousarray(
        np.repeat(wl_v.reshape(HC, 128).T[:, :, None], 32, axis=2))
    wlneg = np.ascontiguousarray(
        np.repeat(-2.0 * wl_v[128:256][:, None], 32, axis=1))
    in_maps = []
    for i in range(N_CORES):
        in_maps.append({
            "query": np.ascontiguousarray(inputs["query"][i], np.float32),
            "memory": np.ascontiguousarray(inputs["memory"][i], np.float32),
            "mask": np.ascontiguousarray(inputs["mask"][i]).astype(np.uint8),
            "wqt": wqt,
            "wkt": wkt,
            "bqk": bqk,
            "wlrep": wlrep,
            "wlneg": wlneg,
        })
    return in_maps


def kernel(**inputs):
    from concourse.bass_utils import run_bass_kernel_spmd

    nc = _get_nc()
    in_maps = make_in_maps(inputs)
    res = run_bass_kernel_spmd(nc, in_maps, list(range(N_CORES)))
    attns = np.stack([res.results[i]["attns"] for i in range(N_CORES)])
    weights = np.stack([res.results[i]["weights"] for i in range(N_CORES)])
    return attns, weights
